# revision 17
# baseline (speedup 1.0000x reference)
"""nn_CRFLayer: CRF Viterbi decode on 8 Trainium2 NeuronCores.

Data parallel over batch: each core decodes 64 of the 512 sequences.
Self-contained: hardcodes B=512, T=512, D=48, n_cores=8.

Per-core kernel (Tile-scheduled, VectorE + Pool engines):
  Layout: partitions = (b, half) interleaved (p = 2b+ch); each forward step
  processes scores[128, 24 cur, 48 prev]; alpha is stored rotated per
  partition-half with constants pre-rotated to compensate.

  Forward step: Pool computes scores = trans_rep + alpha (the only 2-input
  arith op the Pool engine supports); VectorE does the segmented reduce_max,
  the alpha update (add emit, copy_predicated on the t<len mask, pair-swap
  stream_shuffle), and an exact first-index argmax (is_ge mask -> bf16
  mult by (prev-64) -> segmented reduce_min) that is software-pipelined two
  steps behind the alpha chain so the in-order VectorE stream never stalls
  on Pool latency. Backpointers (bf16) stream to DRAM.

  Backward: one-hot dot-product chain (scalar_tensor_tensor accumulate,
  last_tag injection at t == L-1 via precomputed masks, is_equal one-hot
  regeneration), then mask t >= L to 0 and cast to int32.

All value-producing float ops are the same single fp32 adds as the
reference, so the decoded tags match bitwise-exactly.
"""


import numpy as np
import ml_dtypes

import concourse.bass as bass
import concourse.mybir as mybir
from concourse.tile import TileContext
from concourse.tile_rust import add_dep_helper

AL = mybir.AluOpType
F32 = mybir.dt.float32
BF16 = mybir.dt.bfloat16
I32 = mybir.dt.int32

D = 48
HALF = 24
BLOC = 64
BIG = 64.0

PAIR_SWAP_MASK = [i ^ 1 for i in range(32)]


def make_consts(trans: np.ndarray, T: int) -> dict[str, np.ndarray]:
    """Host-prepared constant tensors (all tiny; derived from trans + shapes)."""
    trans = np.asarray(trans, dtype=np.float32)
    trans_rep = np.zeros((128, HALF, D), dtype=np.float32)
    iota_neg = np.zeros((128, HALF, D), dtype=np.float32)
    for ch in range(2):
        prev = (np.arange(D) + HALF * ch) % D
        cur = HALF * ch + np.arange(HALF)
        block = trans[prev][:, cur].T  # [c24, j]
        for b in range(BLOC):
            p = 2 * b + ch
            trans_rep[p] = block
            iota_neg[p] = prev[None, :] - BIG
    iota_t = np.broadcast_to(np.arange(T, dtype=np.float32)[None, :], (128, T)).copy()
    iota48_nat = np.broadcast_to(np.arange(D, dtype=np.float32)[None, :], (BLOC, D)).copy()
    iota_big_nat = iota48_nat + BIG
    return {
        "trans_rep": trans_rep.reshape(128, HALF * D),
        "iota_neg": iota_neg.reshape(128, HALF * D).astype(ml_dtypes.bfloat16),
        "iota_t_il": iota_t,                      # [128, T] f32 (rows 0..127)
        "iota_t_nat": iota_t[:BLOC].copy(),       # [64, T] f32
        "iota48_nat": iota48_nat,                 # [64, 48] f32
        "iota_big_nat": iota_big_nat,             # [64, 48] f32
    }


def make_core_inputs(logits_core, sent_lengths_core, consts) -> dict[str, np.ndarray]:
    L = np.asarray(sent_lengths_core, dtype=np.float32)
    lg = np.asarray(logits_core, dtype=np.float32)
    T = lg.shape[1]
    lg_il = lg.reshape(BLOC, T, 2, HALF).transpose(0, 2, 1, 3).reshape(128, T, HALF)
    return dict(
        consts,
        logits_il=np.ascontiguousarray(lg_il),
        l_il=np.repeat(L, 2).reshape(128, 1),
        l_nat=L.reshape(BLOC, 1),
    )


def crf_kernel(tc: TileContext, outs, ins, T: int = 512, CK: int = 16, CKB: int = 32, repeat: int = 1,
               CSPL: int = 24):
    nc = tc.nc
    logits_il = ins["logits_il"]      # [128, T, 24] dram f32 (p = 2b+h interleaved)
    tags_out = outs["tags"]           # [64, T] dram i32

    bp_dram_il = nc.dram_tensor("bp_scratch", [128, T, HALF], BF16, kind="Internal").ap()
    alpha_dram = nc.dram_tensor("alpha_scratch", [BLOC, 2, D], F32, kind="Internal").ap()

    import contextlib
    with (
        tc.tile_pool(name="persist", bufs=1) as pp,
        tc.tile_pool(name="chunks", bufs=3) as cp,
        tc.tile_pool(name="work", bufs=4) as wp,
        tc.tile_pool(name="pscore", bufs=2, space=bass.MemorySpace.PSUM) as psp,
        tc.For_i(0, repeat, 1) if repeat > 1 else contextlib.nullcontext(),
    ):
        # ---- persistent constants ----
        trans_rep = pp.tile([128, HALF, D], F32, tag="trans_rep")
        nc.sync.dma_start(trans_rep[:].rearrange("p a b -> p (a b)"), ins["trans_rep"])
        iota_neg = pp.tile([128, HALF, D], BF16, tag="iota_neg")
        nc.sync.dma_start(iota_neg[:].rearrange("p a b -> p (a b)"), ins["iota_neg"])
        iota48_nat = pp.tile([BLOC, D], F32, tag="iota48_nat")
        nc.sync.dma_start(iota48_nat[:], ins["iota48_nat"])
        iota_big_nat = pp.tile([BLOC, D], F32, tag="iota_big_nat")
        nc.sync.dma_start(iota_big_nat[:], ins["iota_big_nat"])
        l_il = pp.tile([128, 1], F32, tag="l_il")
        nc.sync.dma_start(l_il[:], ins["l_il"])
        l_nat = pp.tile([BLOC, 1], F32, tag="l_nat")
        nc.sync.dma_start(l_nat[:], ins["l_nat"])
        iota_t_il = pp.tile([128, T], F32, tag="iota_t_il")
        nc.sync.dma_start(iota_t_il[:], ins["iota_t_il"])
        iota_t_nat = pp.tile([BLOC, T], F32, tag="iota_t_nat")
        nc.sync.dma_start(iota_t_nat[:], ins["iota_t_nat"])

        # ---- derived masks ----
        valid_il = pp.tile([128, T], mybir.dt.uint8, tag="valid_il")  # t < L[b]
        nc.vector.tensor_scalar(
            out=valid_il[:], in0=iota_t_il[:], scalar1=l_il[:, 0:1], scalar2=None,
            op0=AL.is_lt,
        )
        # ---- forward state ----
        alpha = pp.tile([128, D], F32, tag="alpha")            # [own(24) | other(24)]
        nc.sync.dma_start(alpha[:, 0:HALF], logits_il[:, 0, :])
        nc.vector.stream_shuffle(alpha[:, HALF:D], alpha[:, 0:HALF], mask=PAIR_SWAP_MASK)

        # ---- forward scan ----
        # Contention-aware schedule: the Pool scores-add overlaps ONLY the
        # bf16-light DVE ops (mult+min of the previous step — measured immune
        # to SBUF-port contention), while the byte-heavy fp32 is_le runs in
        # the Pool-idle window between redmax and the shuffle (the shuffle is
        # what unlocks the next Pool add).
        LAG = 1
        pend = []            # [(mask_tile, bp_ch_tile, slot, chunk_id), ...]
        chunk_left = {}      # chunk_id -> outstanding reduce_min count
        chunk_dma = {}       # chunk_id -> (bp_ch_tile, lo, hi)

        last_sh = [None]

        def emit_mult_min(entry):
            pmask, pbp, pk, cid = entry
            f = wp.tile([128, HALF, D], BF16, tag="f")
            p4i = nc.vector.tensor_tensor(
                out=f[:], in0=pmask[:], in1=iota_neg[:], op=AL.mult,
            )
            if last_sh[0] is not None:
                add_dep_helper(p4i.ins, last_sh[0].ins, sync=False,
                               reason="keep bf16 bp ops under the Pool add")
            nc.vector.tensor_reduce(
                out=pbp[:, pk, :], in_=f[:], axis=mybir.AxisListType.X, op=AL.min,
            )
            chunk_left[cid] -= 1
            if chunk_left[cid] == 0:
                pbp2, lo, hi = chunk_dma[cid]
                nc.sync.dma_start(
                    bp_dram_il[:, lo:hi, :], pbp2[:, lo - cid:hi - cid, :],
                )

        for t0 in range(0, T, CK):
            ck = min(CK, T - t0)
            emit_ch = cp.tile([128, CK, HALF], F32, tag="emit_ch")
            nc.sync.dma_start(emit_ch[:, 0:ck, :], logits_il[:, t0:t0 + ck, :])
            bp_ch = cp.tile([128, CK, HALF], BF16, tag="bp_ch")
            cid = t0
            lo = max(t0, 1)
            chunk_left[cid] = (t0 + ck) - lo
            chunk_dma[cid] = (bp_ch, lo, t0 + ck)
            for t in range(lo, t0 + ck):
                k = t - t0
                scores = wp.tile([128, HALF, D], F32, tag="scores")
                maxv = wp.tile([128, HALF], F32, tag="maxv")
                mask = wp.tile([128, HALF, D], BF16, tag="mask")
                u = wp.tile([128, HALF], F32, tag="u")

                alpha_b = alpha[:].unsqueeze(1).broadcast_to([128, HALF, D])
                nc.gpsimd.tensor_tensor(out=scores[:], in0=trans_rep[:], in1=alpha_b, op=AL.add)
                # bf16 bp work of the previous step runs under the Pool add
                while len(pend) >= LAG:
                    emit_mult_min(pend.pop(0))
                nc.vector.tensor_reduce(
                    out=maxv[:], in_=scores[:], axis=mybir.AxisListType.X, op=AL.max,
                )
                # is_le with the broadcast operand as src0: same mask as
                # is_ge(scores, maxv); placed before the shuffle so it runs
                # while the Pool engine is still idle.
                maxv_b = maxv[:].unsqueeze(2).broadcast_to([128, HALF, D])
                p3i = nc.vector.tensor_tensor(
                    out=mask[:], in0=maxv_b, in1=scores[:], op=AL.is_le,
                )
                nc.vector.tensor_add(out=u[:], in0=maxv[:], in1=emit_ch[:, k, :])
                valid_b = valid_il[:, t:t + 1].broadcast_to([128, HALF])
                nc.vector.copy_predicated(out=alpha[:, 0:HALF], mask=valid_b, data=u[:])
                last_sh[0] = nc.vector.stream_shuffle(
                    alpha[:, HALF:D], alpha[:, 0:HALF], mask=PAIR_SWAP_MASK,
                )
                add_dep_helper(last_sh[0].ins, p3i.ins, sync=False,
                               reason="shuffle (and thus next Pool add) after is_le")
                pend.append((mask, bp_ch, k, cid))
        while pend:
            emit_mult_min(pend.pop(0))

        # ---- last_tag from final alpha ----
        # write alpha (even partitions hold natural alpha[b, 0:48]) to DRAM, read back
        nc.sync.dma_start(alpha_dram.rearrange("b h d -> (b h) d"), alpha[:])
        alpha_nat = pp.tile([BLOC, D], F32, tag="alpha_nat")
        nc.sync.dma_start(alpha_nat[:], alpha_dram[:, 0, :])
        amax = pp.tile([BLOC, 1], F32, tag="amax")
        nc.vector.tensor_reduce(
            out=amax[:], in_=alpha_nat[:], axis=mybir.AxisListType.X, op=AL.max,
        )
        amask = pp.tile([BLOC, D], F32, tag="amask")
        nc.vector.tensor_scalar(
            out=amask[:], in0=alpha_nat[:], scalar1=amax[:, 0:1], scalar2=None,
            op0=AL.is_ge,
        )
        af = pp.tile([BLOC, D], F32, tag="af")
        nc.vector.scalar_tensor_tensor(
            out=af[:], in0=amask[:], scalar=-BIG, in1=iota_big_nat[:],
            op0=AL.mult, op1=AL.add,
        )
        last_tag = pp.tile([BLOC, 1], F32, tag="last_tag")
        nc.vector.tensor_reduce(
            out=last_tag[:], in_=af[:], axis=mybir.AxisListType.X, op=AL.min,
        )

        # ---- backward-pass masks ----
        lm1 = pp.tile([BLOC, 1], F32, tag="lm1")
        nc.vector.tensor_scalar(
            out=lm1[:], in0=l_nat[:], scalar1=-1.0, scalar2=None, op0=AL.add,
        )
        inj = pp.tile([BLOC, T], F32, tag="inj")        # t == L-1
        nc.vector.tensor_scalar(
            out=inj[:], in0=iota_t_nat[:], scalar1=lm1[:, 0:1], scalar2=None,
            op0=AL.is_equal,
        )
        omj = pp.tile([BLOC, T], F32, tag="omj")        # 1 - inj
        nc.vector.tensor_scalar(
            out=omj[:], in0=inj[:], scalar1=-1.0, scalar2=1.0, op0=AL.mult, op1=AL.add,
        )
        lt_inj = pp.tile([BLOC, T], F32, tag="lt_inj")  # inj * last_tag
        nc.vector.tensor_scalar(
            out=lt_inj[:], in0=inj[:], scalar1=last_tag[:, 0:1], scalar2=None,
            op0=AL.mult,
        )
        ltinj2 = pp.tile([BLOC, T], F32, tag="ltinj2")   # BIG*omj + lt_inj
        nc.vector.scalar_tensor_tensor(
            out=ltinj2[:], in0=omj[:], scalar=BIG, in1=lt_inj[:],
            op0=AL.mult, op1=AL.add,
        )
        valid_nat = pp.tile([BLOC, T], F32, tag="valid_nat")
        nc.vector.tensor_scalar(
            out=valid_nat[:], in0=iota_t_nat[:], scalar1=l_nat[:, 0:1], scalar2=None,
            op0=AL.is_lt,
        )

        # ---- backward chain ----
        tagsq = pp.tile([BLOC, T], F32, tag="tagsq")
        h = pp.tile([BLOC, D], F32, tag="h")
        m = pp.tile([BLOC, 1], F32, tag="m")
        junk = pp.tile([BLOC, D], F32, tag="junk")

        # t = T-1 init: q = 0*omj + lt_inj ; h = (iota48 == q)
        nc.vector.memset(m[:], 0.0)
        nc.vector.scalar_tensor_tensor(
            out=tagsq[:, T - 1:T], in0=m[:], scalar=omj[:, T - 1:T],
            in1=ltinj2[:, T - 1:T], op0=AL.mult, op1=AL.add,
        )
        nc.vector.tensor_scalar(
            out=h[:], in0=iota48_nat[:], scalar1=tagsq[:, T - 1:T], scalar2=None,
            op0=AL.is_equal,
        )

        t_hi = T - 1  # highest bp index used is T-1
        for c0 in range(t_hi, 0, -CKB):
            ckb = min(CKB, c0)     # bp indices c0, c0-1, ..., c0-ckb+1 (>=1)
            bpb = cp.tile([BLOC, CKB, D], BF16, tag="bpb")
            bp_v = bp_dram_il.rearrange("(b h) t c -> b h t c", h=2)
            for hh in range(2):
                nc.sync.dma_start(
                    bpb[:, 0:ckb, HALF * hh:HALF * (hh + 1)],
                    bp_v[:, hh, c0 - ckb + 1:c0 + 1, :],
                )
            for tp1 in range(c0, c0 - ckb, -1):
                t = tp1 - 1
                kk = tp1 - (c0 - ckb + 1)
                nc.vector.scalar_tensor_tensor(
                    out=junk[:], in0=bpb[:, kk, :], scalar=1.0, in1=h[:],
                    op0=AL.mult, op1=AL.mult, accum_out=m[:],
                )
                nc.vector.scalar_tensor_tensor(
                    out=tagsq[:, t:t + 1], in0=m[:], scalar=omj[:, t:t + 1],
                    in1=ltinj2[:, t:t + 1], op0=AL.mult, op1=AL.add,
                )
                if t > 0:
                    nc.vector.tensor_scalar(
                        out=h[:], in0=iota48_nat[:], scalar1=tagsq[:, t:t + 1],
                        scalar2=None, op0=AL.is_equal,
                    )

        # ---- final masking + cast + store ----
        tags_f = pp.tile([BLOC, T], F32, tag="tags_f")
        nc.vector.tensor_mul(out=tags_f[:], in0=tagsq[:], in1=valid_nat[:])
        tags_i = pp.tile([BLOC, T], I32, tag="tags_i")
        nc.vector.tensor_copy(out=tags_i[:], in_=tags_f[:])
        nc.sync.dma_start(tags_out, tags_i[:])


# ---------------------------------------------------------------------------
# self-contained harness: build once, shard, run SPMD on 8 cores, unshard
# ---------------------------------------------------------------------------
import concourse.bacc as bacc
from concourse.bass_utils import run_bass_kernel_spmd

B = 512
T = 512
N_CORES = 8


def _input_specs():
    return {
        "logits_il": ([128, T, HALF], F32),
        "l_il": ([128, 1], F32),
        "l_nat": ([BLOC, 1], F32),
        "trans_rep": ([128, HALF * D], F32),
        "iota_neg": ([128, HALF * D], BF16),
        "iota_t_il": ([128, T], F32),
        "iota_t_nat": ([BLOC, T], F32),
        "iota48_nat": ([BLOC, D], F32),
        "iota_big_nat": ([BLOC, D], F32),
    }


_NC = None


def _build_nc():
    global _NC
    if _NC is not None:
        return _NC
    nc = bacc.Bacc(
        "TRN2",
        target_bir_lowering=False,
        debug=False,
        enable_asserts=True,
        num_devices=N_CORES,
    )
    ins = {
        name: nc.dram_tensor(name, shape, dt, kind="ExternalInput").ap()
        for name, (shape, dt) in _input_specs().items()
    }
    outs = {"tags": nc.dram_tensor("tags", [BLOC, T], I32, kind="ExternalOutput").ap()}
    with TileContext(nc) as tc:
        crf_kernel(tc, outs, ins, T=T)
    nc.compile()
    _NC = nc
    return nc


def kernel(logits, sent_lengths, crf_params):
    logits = np.asarray(logits, dtype=np.float32)
    sent_lengths = np.asarray(sent_lengths)
    consts = make_consts(crf_params, T)

    nc = _build_nc()
    in_maps = []
    for core in range(N_CORES):
        lg = logits[core * BLOC:(core + 1) * BLOC]
        sl = sent_lengths[core * BLOC:(core + 1) * BLOC]
        in_maps.append(make_core_inputs(lg, sl, consts))

    br = run_bass_kernel_spmd(nc, in_maps, core_ids=list(range(N_CORES)))
    out = np.concatenate(
        [br.results[core]["tags"] for core in range(N_CORES)], axis=0
    )
    return out.astype(np.int32)

